# revision 1
# baseline (speedup 1.0000x reference)
"""Trainium2 Bass kernel for CustomISTFT (N_FFT=4096, HOP=1024, T=4096 frames).

Per core (frames sharded 512/core across 8 cores):
  Cooley-Tukey split of the 4096-point inverse DFT: j = 64*j1 + j2,
  n = m1 + 64*m2.  Stage 1 contracts j1 (Hermitian extension of the
  one-sided spectrum + twiddle mu^{m1 j2} folded into host-built weights);
  stage 2 contracts j2 (window * 4096/3 normalization folded into weights).
  Only the real output channel needs the full transform; the imaginary
  channel is exactly win[n]*(b0[t] + (-1)^n b2048[t])/4096 (rank-2 per
  parity), computed with K=8 matmuls that also perform its overlap-add.
  The FFT corner-turn (m1 <-> j2) and the OLA layout-turn both go through
  bf16 DRAM round trips with strided reads.  The overlap-add runs on the
  vector engine in [n mod 128, n div 128, t] layout so all shifts are in
  the free dimension.  Host: shard, gather, reorder, halo-add between
  neighbor cores, exact wsum correction on the two edge blocks.
"""

import numpy as np
import ml_dtypes

N_FFT = 4096
HOP = 1024
FREQ = 2049
T_FRAMES = 4096
N_CORES = 8
T_CORE = T_FRAMES // N_CORES  # 512
L_FULL = (T_FRAMES - 1) * HOP + N_FFT
OUT_LEN = L_FULL - N_FFT

_bf16 = ml_dtypes.bfloat16


# ---------------------------------------------------------------- weights
def canonical_rows(j2):
    """(c, k) input rows consumed by the stage-1 call group of column j2.
    None entries are unused (zero-weighted) padding rows."""
    if j2 == 0:
        return [(0, 64 * j1) for j1 in range(33)] + [(1, 64 * j1) for j1 in range(33)]
    if j2 == 32:
        return [(0, 32 + 64 * j1) for j1 in range(32)] + [
            (1, 32 + 64 * j1) for j1 in range(32)
        ]
    if j2 > 32:
        return canonical_rows(64 - j2)
    return (
        [(0, j2 + 64 * j1) for j1 in range(32)]
        + [(1, j2 + 64 * j1) for j1 in range(32)]
        + [(0, (64 - j2) + 64 * j1) for j1 in range(32)]
        + [(1, (64 - j2) + 64 * j1) for j1 in range(32)]
    )


def build_weights(window):
    """w1 [64,128,128] f32 (rows follow canonical_rows, zero-padded),
    w2 [64,128,64] bf16 (window*4096/3 folded), wim [8,1024] bf16."""
    win = window.astype(np.float64)
    mu = np.exp(2j * np.pi / 4096)
    w64c = np.exp(2j * np.pi / 64)
    m1v = np.arange(64)

    w1 = np.zeros((64, 128, 128), dtype=np.float32)
    k_of_call = np.zeros(64, dtype=np.int64)
    for j2 in range(64):
        coef = {}
        for j1 in range(64):
            k = 64 * j1 + j2
            e = w64c ** (m1v * j1)
            if k <= 2048:
                coef[(0, k)] = coef.get((0, k), 0) + e
                coef[(1, k)] = coef.get((1, k), 0) + 1j * e
            else:
                kr = 4096 - k
                coef[(0, kr)] = coef.get((0, kr), 0) + e
                coef[(1, kr)] = coef.get((1, kr), 0) - 1j * e
        tw = mu ** (m1v * j2)
        rows = canonical_rows(j2)
        assert set(rows) == set(coef.keys())
        k_of_call[j2] = len(rows)
        for i, key in enumerate(rows):
            v = coef[key] * tw
            w1[j2, i, :64] = v.real.astype(np.float32)
            w1[j2, i, 64:] = v.imag.astype(np.float32)

    m2v = np.arange(64)
    j2v = np.arange(64)
    ang = 2 * np.pi * np.outer(j2v, m2v) / 64
    c = np.cos(ang) / 4096
    s = np.sin(ang) / 4096
    w2 = np.zeros((64, 128, 64), dtype=np.float64)
    for m1 in range(64):
        n = m1 + 64 * m2v
        wn = win[n] * (4096.0 / 3.0)
        w2[m1, :64, :] = c * wn[None, :]
        w2[m1, 64:, :] = -s * wn[None, :]

    # wim[(2r+par), i] = win[i + 1024 r]/3 * (par == i%2)
    wim = np.zeros((8, 1024), dtype=np.float64)
    iv = np.arange(1024)
    for r in range(4):
        for par in range(2):
            wim[2 * r + par] = (win[iv + 1024 * r] / 3.0) * (iv % 2 == par)
    return w1.astype(_bf16), k_of_call, w2.astype(_bf16), wim.astype(_bf16)


# ---------------------------------------------------------------- device program
def emit_kernel(tc, outre_ap, outim_ap, z_ap, w1_ap, w2_ap, wim_ap, T):
    """Per-core program.  T frames (multiple of 128).
    outre [128, 8, SPAD] f32:  outre[n%128... wait: outre[p, ih, s] =
        sum_r win*x[128*ih + p + 1024*r, s - r]  (real channel, s in [0,T+3))
    outim [SC, 128, 1024] f32: outim[sc, sp, i] = imag channel at block
        s = 128*sc + sp, position i."""
    import concourse.mybir as mybir
    from contextlib import ExitStack

    nc = tc.nc
    dt = mybir.dt
    f32, f32r, bf16 = dt.float32, dt.float32r, dt.bfloat16
    SB = T + 3
    SC = (SB + 127) // 128
    SPAD = outre_ap.shape[2]
    assert SPAD >= SB and outim_ap.shape[0] == SC

    with ExitStack() as ctx:
        const = ctx.enter_context(tc.tile_pool(name="const", bufs=1))

        # ---- weights to SBUF
        w1_sb = const.tile([128, 64, 128], bf16)
        nc.sync.dma_start(w1_sb[:], w1_ap.rearrange("c k m -> k c m"))
        w2_sb = const.tile([128, 64, 64], bf16)
        nc.sync.dma_start(w2_sb[:], w2_ap.rearrange("c k m -> k c m"))
        wim_sb = const.tile([8, 1024], bf16)
        nc.sync.dma_start(wim_sb[:], wim_ap[:])

        dram = ctx.enter_context(tc.tile_pool(name="dram", bufs=1, space="DRAM"))
        a_dram = dram.tile([128, 64, T], bf16)
        x_dram = dram.tile([64, 64, T], bf16)

        # ---- stage 1: gather z rows, matmul, cast to bf16, write A to DRAM
        zpool = ctx.enter_context(tc.tile_pool(name="zt", bufs=4))
        s1ps = ctx.enter_context(tc.tile_pool(name="s1ps", bufs=3, space="PSUM"))
        apool = ctx.enter_context(tc.tile_pool(name="aslot", bufs=4))

        b0t = const.tile([1, T], f32)
        b2t = const.tile([1, T], f32)
        cve = const.tile([1, T], bf16)
        cvo = const.tile([1, T], bf16)

        for p in range(33):
            rows = canonical_rows(p)
            K = len(rows)
            zt = zpool.tile([128, T], f32, tag="zt")
            ofs = 0
            while ofs < K:
                grp = 33 if p == 0 else 32
                c0, kbase = rows[ofs]
                nc.sync.dma_start(
                    zt[ofs : ofs + grp, :],
                    z_ap[c0, kbase::64][:grp, :],
                )
                ofs += grp
            if p == 0:
                # b0 = z[1,0,:] at row 33, b2048 = z[1,2048,:] at row 65
                nc.sync.dma_start(b0t[:], zt[33:34, :])
                nc.sync.dma_start(b2t[:], zt[65:66, :])
                nc.vector.tensor_add(cve[:], b0t[:], b2t[:])
                nc.vector.tensor_sub(cvo[:], b0t[:], b2t[:])
            ztb = zpool.tile([128, T], bf16, tag="ztb")
            nc.any.tensor_copy(ztb[0:K, :], zt[0:K, :])
            calls = [p] if p in (0, 32) else [p, 64 - p]
            for j2 in calls:
                ps = s1ps.tile([128, T], f32, tag="s1ps")
                nc.tensor.matmul(
                    ps[:],
                    w1_sb[0:K, j2, :],
                    ztb[0:K, :],
                    start=True,
                    stop=True,
                )
                aslot = apool.tile([128, T], bf16, tag="aslot")
                nc.any.tensor_copy(aslot[:], ps[:])
                nc.sync.dma_start(a_dram[:, j2, :], aslot[:])

        # ---- corner-turn read + stage 2 + x to DRAM
        rpool = ctx.enter_context(tc.tile_pool(name="rslot", bufs=4))
        s2ps = ctx.enter_context(tc.tile_pool(name="s2ps", bufs=2, space="PSUM"))
        xpool = ctx.enter_context(tc.tile_pool(name="xslot", bufs=4))

        for m1 in range(64):
            rt = rpool.tile([128, T], bf16, tag="rslot")
            for reim in range(2):
                nc.sync.dma_start(
                    rt[64 * reim : 64 * reim + 64, :],
                    a_dram[64 * reim + m1, :, :],
                )
            xps = s2ps.tile([64, T], f32, tag="s2ps")
            nc.tensor.matmul(xps[:], w2_sb[:, m1, :], rt[:], start=True, stop=True)
            xs = xpool.tile([64, T], bf16, tag="xslot")
            nc.any.tensor_copy(xs[:], xps[:])
            nc.sync.dma_start(x_dram[:, m1, :], xs[:])

        # ---- OLA (real): x2[np, nh, t] = x[128*nh + np, t]
        # sig[np, ih, s] = sum_r x2[np, ih + 8r, s - r]
        x2 = const.tile([128, 32, T], bf16)
        nc.sync.dma_start(
            x2[:], x_dram[:].rearrange("(nh par) m1 t -> (par m1) nh t", par=2)
        )
        sig = const.tile([128, 8, SPAD], f32)
        nc.vector.memset(sig[:], 0.0)
        for r in range(4):
            nc.vector.tensor_add(
                sig[:, :, r : r + T],
                sig[:, :, r : r + T],
                x2[:, 8 * r : 8 * r + 8, :],
            )
        nc.sync.dma_start(outre_ap[:], sig[:])

        # ---- imag channel: CS[2r+par, s] = cv_par[s - r]; outim = CS^T @ wim
        cs = const.tile([8, SC * 128], bf16)
        nc.vector.memset(cs[:], 0.0)
        for r in range(4):
            nc.sync.dma_start(cs[2 * r : 2 * r + 1, r : r + T], cve[:])
            nc.sync.dma_start(cs[2 * r + 1 : 2 * r + 2, r : r + T], cvo[:])
        impool = ctx.enter_context(tc.tile_pool(name="imps", bufs=2, space="PSUM"))
        imsb = ctx.enter_context(tc.tile_pool(name="imsb", bufs=2))
        for sc in range(SC):
            it = imsb.tile([128, 1024], f32, tag="imsb")
            for half in range(2):
                ips = impool.tile([128, 512], f32, tag="imps")
                nc.tensor.matmul(
                    ips[:],
                    cs[:, sc * 128 : (sc + 1) * 128],
                    wim_sb[:, 512 * half : 512 * (half + 1)],
                    start=True,
                    stop=True,
                )
                nc.any.tensor_copy(it[:, 512 * half : 512 * (half + 1)], ips[:])
            nc.sync.dma_start(outim_ap[sc], it[:])


# ---------------------------------------------------------------- build + run
_CACHE = {}
SPAD = 520  # padded s extent of outre (>= T_CORE + 3)


def _build(T):
    import concourse.bacc as bacc
    import concourse.tile as tile
    import concourse.mybir as mybir

    dt = mybir.dt
    SC = (T + 3 + 127) // 128
    nc = bacc.Bacc("TRN2", target_bir_lowering=False, debug=False, num_devices=N_CORES)
    z_t = nc.dram_tensor("z", [2, FREQ, T], dt.float32, kind="ExternalInput")
    w1_t = nc.dram_tensor("w1", [64, 128, 128], dt.bfloat16, kind="ExternalInput")
    w2_t = nc.dram_tensor("w2", [64, 128, 64], dt.bfloat16, kind="ExternalInput")
    wim_t = nc.dram_tensor("wim", [8, 1024], dt.bfloat16, kind="ExternalInput")
    spad = max(SPAD, T + 3)
    outre_t = nc.dram_tensor("outre", [128, 8, spad], dt.float32, kind="ExternalOutput")
    outim_t = nc.dram_tensor("outim", [SC, 128, 1024], dt.float32, kind="ExternalOutput")
    with tile.TileContext(nc) as tc:
        emit_kernel(
            tc, outre_t.ap(), outim_t.ap(), z_t.ap(), w1_t.ap(), w2_t.ap(),
            wim_t.ap(), T,
        )
    nc.compile()
    return nc


def core_out_to_sig(outre, outim, T):
    """[128,8,spad] + [SC,128,1024] -> [2, (T+3)*1024] f32."""
    SB = T + 3
    re = outre.transpose(2, 1, 0).reshape(-1, 1024)[:SB]  # [s, i]
    im = outim.reshape(-1, 1024)[:SB]
    return np.stack([re.reshape(-1), im.reshape(-1)])


def kernel(z, window):
    from concourse.bass_utils import run_bass_kernel_spmd

    z = np.asarray(z, dtype=np.float32)
    window = np.asarray(window, dtype=np.float32)
    assert z.shape == (2, FREQ, T_FRAMES)

    if "nc" not in _CACHE:
        _CACHE["nc"] = _build(T_CORE)
    nc = _CACHE["nc"]

    w1, _, w2, wim = build_weights(window)
    in_maps = []
    for m in range(N_CORES):
        zc = np.ascontiguousarray(z[:, :, m * T_CORE : (m + 1) * T_CORE])
        in_maps.append({"z": zc, "w1": w1, "w2": w2, "wim": wim})
    res = run_bass_kernel_spmd(nc, in_maps, core_ids=list(range(N_CORES)))

    full = np.zeros((2, L_FULL), dtype=np.float32)
    span = (T_CORE + 3) * 1024
    for m in range(N_CORES):
        o = core_out_to_sig(res.results[m]["outre"], res.results[m]["outim"], T_CORE)
        full[:, m * T_CORE * HOP : m * T_CORE * HOP + span] += o
    out = full[:, N_FFT // 2 : L_FULL - N_FFT // 2]

    win = window.astype(np.float64)
    ws_start = win[0:1024] + win[1024:2048] + win[2048:3072]
    ws_end = win[1024:2048] + win[2048:3072] + win[3072:4096]
    out[:, :1024] *= ((3.0 / 4096.0) / ws_start).astype(np.float32)[None, :]
    out[:, -1024:] *= ((3.0 / 4096.0) / ws_end).astype(np.float32)[None, :]
    return out

